# revision 9
# baseline (speedup 1.0000x reference)
"""DenseCINConv message-passing kernel for 8 Trainium2 NeuronCores.

Strategy: shard target cells (rows) of each dim across the 8 cores. Per
(branch, core): edges sorted by target block (128 targets), padded per block
to whole 128-edge tiles (tile counts equalized across cores so all cores run
one SPMD program). Sources are remapped to per-segment compacted fp16 tables
(<=32768 rows, int16 dma_gather indices; first-use order makes the gather
quasi-sequential in HBM). Aggregation = per-block selection-matrix matmuls
accumulated in PSUM, feature-major [64, targets]. Activations live in SBUF as
[128, rows/2] fp16 (features x two row-halves). MLP/BatchNorm run in 4 phases
separated by cross-core AllReduces of the BN statistics.
"""
import numpy as np

D = 64
P = 128
N_CORES = 8
BN_EPS = 1e-5
N0, N1, N2 = 100000, 200000, 75000
PANEL = 512
CALL_T = 24          # max tiles (128 edges) per dma_gather call
SEG_MAX = 32700      # max unique rows per compact table (int16 headroom)
XSLICE = 2048        # x_tgt stream slice (cols)

DIMS = [0, 1, 2]
NDIM = {0: N0, 1: N1, 2: N2}
SHARD = {d: NDIM[d] // N_CORES for d in DIMS}            # 12500, 25000, 9375
RPAD = {d: -(-SHARD[d] // (2 * P)) * (2 * P) for d in DIMS}   # 12544, 25088, 9472
HALF = {d: RPAD[d] // 2 for d in DIMS}                   # 6272, 12544, 4736

# branches: (name, tgt_dim, src_dim, index_key)
BRANCHES = [
    ("up0", 0, 0, "up0_index"),
    ("cob0", 0, 1, "cob0_index"),
    ("up1", 1, 1, "up1_index"),
    ("down1", 1, 1, "down1_index"),
    ("bnd1", 1, 0, "bnd1_index"),
    ("cob1", 1, 2, "cob1_index"),
    ("down2", 2, 2, "down2_index"),
    ("bnd2", 2, 1, "bnd2_index"),
]
# combine order per dim (branch indices into BRANCHES)
COMB = {0: [0, 1], 1: [2, 3, 4, 5], 2: [6, 7]}
# param sub-dict and key per branch
BPAR = {"up0": ("d0", "up"), "cob0": ("d0", "cob"), "up1": ("d1", "up"),
        "down1": ("d1", "down"), "bnd1": ("d1", "bnd"), "cob1": ("d1", "cob"),
        "down2": ("d2", "down"), "bnd2": ("d2", "bnd")}


def _first_use_codes(a):
    """Factorize int array by first-use order. Returns codes, uniques."""
    u, first, inv = np.unique(a, return_index=True, return_inverse=True)
    order = np.argsort(first, kind="stable")
    rank = np.empty(len(u), np.int64)
    rank[order] = np.arange(len(u))
    return rank[inv], u[order]


def _host_prep(inputs):
    """Build per-core gather streams / tables / metadata. Returns (meta, per_core_arrays)."""
    xw = {}
    for d, key in ((0, "x0"), (1, "x1"), (2, "x2")):
        x = np.asarray(inputs[key], np.float32)
        w = np.zeros((x.shape[0], 128), np.float16)
        w[:, :D] = x.astype(np.float16)
        xw[d] = w

    meta = []          # per (branch, half): dict with blocks' tile counts, segments, calls
    core_idx = [[] for _ in range(N_CORES)]    # int16 wrapped pieces
    core_rel = [[] for _ in range(N_CORES)]
    core_tabs = [{} for _ in range(N_CORES)]   # name -> array
    tab_shapes = {}

    for bi, (name, dt, ds, ikey) in enumerate(BRANCHES):
        idx = np.asarray(inputs[ikey])
        src_a, tgt_a = idx[0].astype(np.int64), idx[1].astype(np.int64)
        S, H = SHARD[dt], HALF[dt]
        core_of = tgt_a // S
        for half in (0, 1):
            nblk = H // P
            # per core, per block: src lists
            per_core_blocks = []
            for c in range(N_CORES):
                m = core_of == c
                tl = tgt_a[m] - c * S
                sb = src_a[m]
                hm = (tl >= H) == bool(half)
                r = tl[hm] - half * H
                s = sb[hm]
                blk = r // P
                rel = (r % P).astype(np.float32)
                order = np.argsort(blk, kind="stable")
                blk, rel, s = blk[order], rel[order], s[order]
                bounds = np.searchsorted(blk, np.arange(nblk + 1))
                per_core_blocks.append((s, rel, bounds))
            tiles_per_blk = np.zeros(nblk, np.int64)
            for c in range(N_CORES):
                _, _, bounds = per_core_blocks[c]
                cnt = bounds[1:] - bounds[:-1]
                tiles_per_blk = np.maximum(tiles_per_blk, -(-cnt // P))
            tiles_per_blk = np.maximum(tiles_per_blk, 1)

            # build padded streams per core (src full-ids, rel with -1 pads)
            streams = []
            for c in range(N_CORES):
                s, rel, bounds = per_core_blocks[c]
                ss, rr = [], []
                for b in range(nblk):
                    sb = s[bounds[b]:bounds[b + 1]]
                    rb = rel[bounds[b]:bounds[b + 1]]
                    pad = tiles_per_blk[b] * P - len(sb)
                    ss.append(sb); ss.append(np.full(pad, -1, np.int64))
                    rr.append(rb); rr.append(np.full(pad, -1.0, np.float32))
                streams.append((np.concatenate(ss), np.concatenate(rr)))

            # segment blocks so per-segment uniques (plus zero row) <= SEG_MAX for every core
            seg_bounds = [0]
            sets = [set() for _ in range(N_CORES)]
            for b in range(nblk):
                trial = []
                for c in range(N_CORES):
                    s, _, bounds = per_core_blocks[c]
                    trial.append(set(s[bounds[b]:bounds[b + 1]].tolist()))
                if any(len(sets[c] | trial[c]) + 1 > SEG_MAX for c in range(N_CORES)):
                    seg_bounds.append(b)
                    sets = trial
                else:
                    for c in range(N_CORES):
                        sets[c] |= trial[c]
            seg_bounds.append(nblk)

            tile_off = np.concatenate([[0], np.cumsum(tiles_per_blk)])
            segs = []
            for si in range(len(seg_bounds) - 1):
                b0, b1 = seg_bounds[si], seg_bounds[si + 1]
                t0, t1 = int(tile_off[b0]), int(tile_off[b1])
                tabname = f"tab_{bi}_{half}_{si}"
                umax = 0
                codes_per_core = []
                for c in range(N_CORES):
                    s_pad, _ = streams[c]
                    seg_src = s_pad[t0 * P:t1 * P]
                    realm = seg_src >= 0
                    codes = np.zeros(len(seg_src), np.int64)
                    if realm.any():
                        cds, uniq = _first_use_codes(seg_src[realm])
                        codes[realm] = cds + 1
                        tab = np.zeros((len(uniq) + 1, 128), np.float16)
                        tab[1:] = xw[ds][uniq]
                    else:
                        tab = np.zeros((1, 128), np.float16)
                    umax = max(umax, tab.shape[0])
                    codes_per_core.append(codes)
                    core_tabs[c][tabname] = tab
                for c in range(N_CORES):
                    t = core_tabs[c][tabname]
                    if t.shape[0] < umax:
                        core_tabs[c][tabname] = np.vstack(
                            [t, np.zeros((umax - t.shape[0], 128), np.float16)])
                tab_shapes[tabname] = umax
                # calls: group whole blocks, <= CALL_T tiles each
                calls = []
                cb0 = b0
                while cb0 < b1:
                    cb1 = cb0
                    tsum = 0
                    while cb1 < b1 and tsum + tiles_per_blk[cb1] <= CALL_T:
                        tsum += int(tiles_per_blk[cb1]); cb1 += 1
                    if cb1 == cb0:
                        cb1 = cb0 + 1
                        tsum = int(tiles_per_blk[cb0])
                    calls.append((int(tile_off[cb0]), tsum,
                                  [(int(b), int(tiles_per_blk[b])) for b in range(cb0, cb1)]))
                    cb0 = cb1
                segs.append({"name": tabname, "t0": t0, "t1": t1, "calls": calls,
                             "codes": codes_per_core})
            # append per-core wrapped idx / rel
            for c in range(N_CORES):
                s_pad, r_pad = streams[c]
                ntile = len(s_pad) // P
                codes_all = np.zeros(len(s_pad), np.int16)
                for sg in segs:
                    codes_all[sg["t0"] * P:sg["t1"] * P] = \
                        sg["codes"][c].astype(np.int16)
                wrapped = codes_all.reshape(-1, 16).T      # [16, L/16]
                core_idx[c].append(np.tile(wrapped, (8, 1)))
                core_rel[c].append(r_pad.astype(np.float16).reshape(ntile, P).T)
            for sg in segs:
                del sg["codes"]
            meta.append({"bi": bi, "dt": dt, "half": half, "nblk": nblk,
                         "tiles_per_blk": tiles_per_blk.tolist(), "segs": segs,
                         "ntiles": int(tile_off[-1])})

    idx_all = [np.concatenate(core_idx[c], axis=1) for c in range(N_CORES)]
    rel_all = [np.concatenate(core_rel[c], axis=1) for c in range(N_CORES)]

    # x_tgt device layout [128, H0+H1+H2] fp16 (per dim: top rows feats 0-63, bottom 64-127)
    xoff = {}
    off = 0
    for d in DIMS:
        xoff[d] = off
        off += HALF[d]
    xt = [np.zeros((128, off), np.float16) for _ in range(N_CORES)]
    for c in range(N_CORES):
        for d, key in ((0, "x0"), (1, "x1"), (2, "x2")):
            x = np.asarray(inputs[key], np.float32).astype(np.float16)
            S, H, R = SHARD[d], HALF[d], RPAD[d]
            shard = np.zeros((R, D), np.float16)
            shard[:S] = x[c * S:(c + 1) * S]
            xt[c][0:D, xoff[d]:xoff[d] + H] = shard[:H].T
            xt[c][D:2 * D, xoff[d]:xoff[d] + H] = shard[H:].T

    # weights / vectors
    params = inputs["params"]
    wts, vecs = [], []
    wmap, vmap = {}, {}

    def addw(w):
        wts.append(np.vstack([w, w]).astype(np.float16))
        return len(wts) - 1

    def addv(v):
        vecs.append(np.concatenate([v, v]).astype(np.float32))
        return len(vecs) - 1

    for bi, (name, dt, ds, _) in enumerate(BRANCHES):
        dk, bk = BPAR[name]
        p = {k: np.asarray(v, np.float32) for k, v in params[dk][bk].items()}
        wmap[(bi, 1)] = addw(p["W1"])
        wmap[(bi, 2)] = addw(p["W2"])
        for k in ("b1", "g1", "bt1", "b2", "g2", "bt2"):
            vmap[(bi, k)] = addv(p[k])
    for d in DIMS:
        pc = {k: np.asarray(v, np.float32) for k, v in params[f"d{d}"]["comb"].items()}
        for j, bi in enumerate(COMB[d]):
            wmap[("c", bi)] = addw(pc["Wc"][j * D:(j + 1) * D])
        for k in ("bc", "gc", "btc"):
            vmap[(d, k)] = addv(pc[k])
    wts_arr = np.stack(wts).transpose(1, 0, 2).copy()  # [128, NW, 64] fp16
    vecs_arr = np.stack(vecs).reshape(len(vecs), 128).T.copy()  # [128, NV] fp32

    iota = np.tile(np.arange(P, dtype=np.float16), (P, 1))

    return {
        "meta": meta, "tab_shapes": tab_shapes, "xoff": xoff,
        "wts": wts_arr, "vecs": vecs_arr, "wmap": wmap, "vmap": vmap,
        "iota": iota, "idx_all": idx_all, "rel_all": rel_all, "xt": xt,
        "core_tabs": core_tabs,
    }


def _emit_allreduce(nc, mybir, cin, cout, sem):
    with nc.Block() as block:
        @block.gpsimd
        def _(g):
            g.collective_compute(
                "AllReduce", mybir.AluOpType.add,
                replica_groups=[list(range(N_CORES))],
                ins=[cin[:]], outs=[cout[:]]).then_inc(sem, 1)
            g.wait_ge(sem, 1)


def _build_program(prep):
    import concourse.bacc as bacc
    import concourse.tile as tile
    from concourse import mybir
    from contextlib import ExitStack

    fp16, fp32, i16 = mybir.dt.float16, mybir.dt.float32, mybir.dt.int16
    AF = mybir.ActivationFunctionType
    OP = mybir.AluOpType

    meta = prep["meta"]
    TOT_TILES = sum(m["ntiles"] for m in meta)
    NW = prep["wts"].shape[1]
    NV = prep["vecs"].shape[1]
    XTOT = sum(HALF[d] for d in DIMS)
    xoff = prep["xoff"]

    nc = bacc.Bacc("TRN2", target_bir_lowering=False)

    # ---- DRAM params
    tabs_e = {n: nc.declare_dram_parameter(n, [u, 128], fp16, isOutput=False)
              for n, u in prep["tab_shapes"].items()}
    idx_e = nc.declare_dram_parameter("idx_all", [128, TOT_TILES * 8], i16, isOutput=False)
    rel_e = nc.declare_dram_parameter("rel_all", [128, TOT_TILES], fp16, isOutput=False)
    xt_e = nc.declare_dram_parameter("xt", [128, XTOT], fp16, isOutput=False)
    wts_e = nc.declare_dram_parameter("wts", [128, NW, D], fp16, isOutput=False)
    vecs_e = nc.declare_dram_parameter("vecs", [128, NV], fp32, isOutput=False)
    iota_e = nc.declare_dram_parameter("iota", [P, P], fp16, isOutput=False)
    out_e = {d: nc.declare_dram_parameter(f"out{d}", [128, HALF[d]], fp32, isOutput=True)
             for d in DIMS}

    cc_in = [nc.dram_tensor(f"cc_in{k}", [128, 16], fp32) for k in range(3)]
    cc_out = [nc.dram_tensor(f"cc_out{k}", [128, 16], fp32, addr_space="Shared")
              for k in range(3)]
    comb_dram = [nc.dram_tensor(f"comb{k}", [D, 16], fp32) for k in range(3)]

    # ---- persistent SBUF
    # acts: per branch a region of HALF[dt] cols
    aoff = {}
    off = 0
    for bi, (nm, dt, _, _) in enumerate(BRANCHES):
        aoff[bi] = off
        off += HALF[dt]
    ATOT = off
    acts = nc.alloc_sbuf_tensor("acts", [128, ATOT], fp16)
    rel_t = nc.alloc_sbuf_tensor("rel_t", [128, TOT_TILES], fp16)
    iota_t = nc.alloc_sbuf_tensor("iota_t", [P, P], fp16)
    wts_t = nc.alloc_sbuf_tensor("wts_t", [128, NW, D], fp16)
    vecs_t = nc.alloc_sbuf_tensor("vecs_t", [128, NV], fp32)

    # panels per branch: list of (col0, width); panel grid shared by layers
    panels = {}
    NPAN = 0
    pan_col = {}      # (bi) -> start index into stat columns
    for bi, (nm, dt, _, _) in enumerate(BRANCHES):
        H = HALF[dt]
        ps = []
        c0 = 0
        while c0 < H:
            w = min(PANEL, H - c0)
            ps.append((c0, w))
            c0 += w
        panels[bi] = ps
        pan_col[bi] = NPAN
        NPAN += len(ps)
    # combine (dim) panels use branch0 regions
    dpan_col = {}
    NPANC = 0
    for d in DIMS:
        dpan_col[d] = NPANC
        NPANC += len(panels[COMB[d][0]])

    sums1 = nc.alloc_sbuf_tensor("sums1", [128, NPAN], fp32)
    bn61 = nc.alloc_sbuf_tensor("bn61", [128, 2 * NPAN, 3], fp32)
    sums2 = nc.alloc_sbuf_tensor("sums2", [128, NPAN], fp32)
    bn62 = nc.alloc_sbuf_tensor("bn62", [128, 2 * NPAN, 3], fp32)
    sums3 = nc.alloc_sbuf_tensor("sums3", [128, NPANC], fp32)
    bn63 = nc.alloc_sbuf_tensor("bn63", [128, 2 * NPANC, 3], fp32)
    stats = [nc.alloc_sbuf_tensor(f"stats{k}", [128, 16], fp32) for k in range(3)]
    epsv = nc.alloc_sbuf_tensor("epsv", [128, 1], fp32)
    fakeb = nc.alloc_sbuf_tensor("fakeb", [128, 8], fp16)   # per-branch fake activation
    fakec = nc.alloc_sbuf_tensor("fakec", [128, 3], fp16)   # per-dim fake zc
    sb = [nc.alloc_sbuf_tensor(f"sb{k}", [128, 16], fp32) for k in range(3)]

    cc_sems = [nc.alloc_semaphore(f"ccs{k}") for k in range(3)]

    def stat_reduce(tc, pool, sums_t, bn6_t, stats_t, col, p0, p1, cnts_t):
        """reduce per-panel stats [p0,p1) -> stats_t[:, col(2)] = (sum, sumsq)."""
        nc.vector.tensor_reduce(out=stats_t[:, col:col + 1], in_=sums_t[:, p0:p1],
                                axis=mybir.AxisListType.X, op=OP.add)
        musq = pool.tile([128, 2 * (p1 - p0)], fp32, tag="musq")
        nc.vector.tensor_tensor(out=musq[:], in0=bn6_t[:, 2 * p0:2 * p1, 1],
                                in1=bn6_t[:, 2 * p0:2 * p1, 1], op=OP.mult)
        nc.vector.tensor_tensor(out=musq[:], in0=musq[:],
                                in1=cnts_t[:, 2 * p0:2 * p1], op=OP.mult)
        t2 = pool.tile([128, 2], fp32, tag="sqp")
        nc.vector.tensor_reduce(out=t2[:, 0:1], in_=bn6_t[:, 2 * p0:2 * p1, 2],
                                axis=mybir.AxisListType.X, op=OP.add)
        nc.vector.tensor_reduce(out=t2[:, 1:2], in_=musq[:],
                                axis=mybir.AxisListType.X, op=OP.add)
        nc.vector.tensor_tensor(out=stats_t[:, col + 1:col + 2], in0=t2[:, 0:1],
                                in1=t2[:, 1:2], op=OP.add)

    def stats_to_scale(tc, pool, k, entries):
        """After allreduce k: load cc_out, combine halves, compute scale/bias into sb[k].

        entries: (col, Ntrue, g_vec_idx, bt_vec_idx, fake_ap, nfake); fake_ap is a
        [128,1] fp16 column holding the layer value fake (padded) rows carry; its
        contribution is removed from the all-reduced sums before mean/var."""
        top = pool.tile([D, 16], fp32, tag="cctop")
        bot = pool.tile([D, 16], fp32, tag="ccbot")
        nc.gpsimd.dma_start(out=top[:], in_=cc_out[k][0:D, :])
        nc.gpsimd.dma_start(out=bot[:], in_=cc_out[k][D:2 * D, :])
        nc.vector.tensor_tensor(out=top[:], in0=top[:], in1=bot[:], op=OP.add)
        nc.gpsimd.dma_start(out=comb_dram[k][:], in_=top[:])
        nc.gpsimd.dma_start(out=stats[k][0:D, :], in_=comb_dram[k][:])
        nc.gpsimd.dma_start(out=stats[k][D:2 * D, :], in_=comb_dram[k][:])
        for (col, Ntrue, gi, bti, fake_ap, nfake) in entries:
            mean = pool.tile([128, 1], fp32, tag="mean")
            e2 = pool.tile([128, 1], fp32, tag="e2")
            if nfake:
                sumc = pool.tile([128, 1], fp32, tag="sumc")
                nc.vector.scalar_tensor_tensor(
                    out=sumc[:], in0=fake_ap, scalar=-float(nfake),
                    in1=stats[k][:, col:col + 1], op0=OP.mult, op1=OP.add)
                fsq = pool.tile([128, 1], fp32, tag="fsq")
                nc.vector.tensor_tensor(out=fsq[:], in0=fake_ap, in1=fake_ap, op=OP.mult)
                sqc = pool.tile([128, 1], fp32, tag="sqc")
                nc.vector.scalar_tensor_tensor(
                    out=sqc[:], in0=fsq[:], scalar=-float(nfake),
                    in1=stats[k][:, col + 1:col + 2], op0=OP.mult, op1=OP.add)
                nc.vector.tensor_scalar_mul(mean[:], sumc[:], 1.0 / Ntrue)
                nc.vector.tensor_scalar_mul(e2[:], sqc[:], 1.0 / Ntrue)
            else:
                nc.vector.tensor_scalar_mul(mean[:], stats[k][:, col:col + 1], 1.0 / Ntrue)
                nc.vector.tensor_scalar_mul(e2[:], stats[k][:, col + 1:col + 2], 1.0 / Ntrue)
            var = pool.tile([128, 1], fp32, tag="var")
            nc.vector.tensor_tensor(out=var[:], in0=mean[:], in1=mean[:], op=OP.mult)
            nc.vector.tensor_tensor(out=var[:], in0=e2[:], in1=var[:], op=OP.subtract)
            nc.vector.tensor_scalar_max(var[:], var[:], 0.0)
            sig = pool.tile([128, 1], fp32, tag="sig")
            nc.scalar.activation(out=sig[:], in_=var[:], func=AF.Sqrt, bias=epsv[:, 0:1], scale=1.0)
            inv = pool.tile([128, 1], fp32, tag="inv")
            nc.vector.reciprocal(out=inv[:], in_=sig[:])
            nc.vector.tensor_tensor(out=sb[k][:, col:col + 1], in0=vecs_t[:, gi:gi + 1],
                                    in1=inv[:], op=OP.mult)
            t = pool.tile([128, 1], fp32, tag="tmp")
            nc.vector.tensor_tensor(out=t[:], in0=mean[:], in1=sb[k][:, col:col + 1],
                                    op=OP.mult)
            nc.vector.tensor_tensor(out=sb[k][:, col + 1:col + 2],
                                    in0=vecs_t[:, bti:bti + 1], in1=t[:], op=OP.subtract)

    wmap, vmap = prep["wmap"], prep["vmap"]

    # cnts constants for stat aggregation (per panel: count of even/odd elems per bn_stats)
    cnts1 = np.zeros((128, 2 * NPAN), np.float32)
    for bi in range(8):
        for pi, (c0, w) in enumerate(panels[bi]):
            j = pan_col[bi] + pi
            cnts1[:, 2 * j] = (w + 1) // 2
            cnts1[:, 2 * j + 1] = w // 2
    cnts3 = np.zeros((128, 2 * NPANC), np.float32)
    for d in DIMS:
        for pi, (c0, w) in enumerate(panels[COMB[d][0]]):
            j = dpan_col[d] + pi
            cnts3[:, 2 * j] = (w + 1) // 2
            cnts3[:, 2 * j + 1] = w // 2
    cnts1_e = nc.declare_dram_parameter("cnts1", [128, 2 * NPAN], fp32, isOutput=False)
    cnts3_e = nc.declare_dram_parameter("cnts3", [128, 2 * NPANC], fp32, isOutput=False)
    cnts1_t = nc.alloc_sbuf_tensor("cnts1_t", [128, 2 * NPAN], fp32)
    cnts3_t = nc.alloc_sbuf_tensor("cnts3_t", [128, 2 * NPANC], fp32)

    # =================== PHASE 1 ===================
    with tile.TileContext(nc) as tc, ExitStack() as ctx:
        gpool = ctx.enter_context(tc.tile_pool(name="g", bufs=3))
        ipool = ctx.enter_context(tc.tile_pool(name="i", bufs=3))
        spool = ctx.enter_context(tc.tile_pool(name="s", bufs=6))
        xpool = ctx.enter_context(tc.tile_pool(name="x", bufs=2))
        scpool = ctx.enter_context(tc.tile_pool(name="sc", bufs=2))
        aps = ctx.enter_context(tc.tile_pool(name="aps", bufs=4, space="PSUM"))
        zps = ctx.enter_context(tc.tile_pool(name="zps", bufs=2, space="PSUM"))
        small = ctx.enter_context(tc.tile_pool(name="sm", bufs=2))

        nc.sync.dma_start(out=rel_t[:], in_=rel_e[:])
        nc.sync.dma_start(out=iota_t[:], in_=iota_e[:])
        nc.sync.dma_start(out=wts_t[:], in_=wts_e[:])
        nc.sync.dma_start(out=vecs_t[:], in_=vecs_e[:])
        nc.sync.dma_start(out=cnts1_t[:], in_=cnts1_e[:])
        nc.sync.dma_start(out=cnts3_t[:], in_=cnts3_e[:])
        nc.vector.memset(epsv[:], BN_EPS)

        tile_base = 0
        for m in meta:
            bi, dt, half = m["bi"], m["dt"], m["half"]
            H = HALF[dt]
            hp = slice(half * D, (half + 1) * D)
            tp = (0, 0) if half == 0 else (0, 64)
            A = aoff[bi]
            tpb = m["tiles_per_blk"]
            # x_tgt slices for this (dim, half): xt[hp, xoff+.. ]
            blk_done = 0
            pan_ready = 0
            xt_tile = None
            xt_lo = -1
            psum_by_blk = {}
            for sg in m["segs"]:
                tab = tabs_e[sg["name"]]
                for (t0, ntiles, blocks) in sg["calls"]:
                    idx_t = ipool.tile([128, CALL_T * 8], i16, tag="idx")
                    nc.sync.dma_start(out=idx_t[:, :ntiles * 8],
                                      in_=idx_e[:, (tile_base + t0) * 8:(tile_base + t0 + ntiles) * 8])
                    g_t = gpool.tile([128, CALL_T, 128], fp16, tag="g")
                    nc.gpsimd.dma_gather(
                        out_ap=g_t[:, :ntiles, :], in_ap=tab[:],
                        idxs_ap=idx_t[:, :ntiles * 8],
                        num_idxs=ntiles * P, num_idxs_reg=ntiles * P,
                        elem_size=128, single_packet=False)
                    kloc = 0
                    for (b, tb) in blocks:
                        ps = aps.tile([128, P], fp32, tag="aggps")
                        for k in range(tb):
                            gcol = tile_base + t0 + kloc + k
                            s_t = spool.tile([P, P], fp16, tag="s")
                            nc.vector.tensor_tensor(
                                out=s_t[:],
                                in0=rel_t[:, gcol:gcol + 1].to_broadcast([P, P]),
                                in1=iota_t[:], op=OP.is_equal)
                            nc.tensor.matmul(out=ps[hp, :], lhsT=g_t[:, kloc + k, 0:D],
                                             rhs=s_t[:], start=(k == 0), stop=(k == tb - 1),
                                             tile_position=tp)
                        kloc += tb
                        # X = agg + x_tgt
                        c0 = b * P
                        if xt_lo < 0 or c0 >= xt_lo + XSLICE:
                            xt_lo = (c0 // XSLICE) * XSLICE
                            w = min(XSLICE, H - xt_lo)
                            xt_tile = xpool.tile([128, XSLICE], fp16, tag="xt")
                            nc.sync.dma_start(out=xt_tile[:, :w],
                                              in_=xt_e[:, xoff[dt] + xt_lo:xoff[dt] + xt_lo + w])
                        nc.vector.tensor_tensor(
                            out=acts[hp, A + c0:A + c0 + P], in0=ps[hp, :],
                            in1=xt_tile[hp, c0 - xt_lo:c0 - xt_lo + P], op=OP.add)
                        blk_done += 1
                        # z1 for completed panels
                        while pan_ready < len(panels[bi]):
                            pc0, pw = panels[bi][pan_ready]
                            if blk_done * P < pc0 + pw:
                                break
                            zp = zps.tile([128, PANEL], fp32, tag="z")
                            wv = wts_t[:, wmap[(bi, 1)], :]
                            nc.tensor.matmul(out=zp[hp, :pw], lhsT=wv[hp, :],
                                             rhs=acts[hp, A + pc0:A + pc0 + pw],
                                             start=True, stop=True,
                                             tile_position=(half * 64, half * 64))
                            j = pan_col[bi] + pan_ready
                            vb1 = vmap[(bi, "b1")]
                            nc.scalar.activation(
                                out=acts[hp, A + pc0:A + pc0 + pw], in_=zp[hp, :pw],
                                func=AF.Identity, bias=vecs_t[hp, vb1:vb1 + 1], scale=1.0,
                                accum_out=sums1[hp, j:j + 1])
                            nc.vector.bn_stats(out=bn61[hp, 2 * j:2 * j + 2, :],
                                               in_=acts[hp, A + pc0:A + pc0 + pw])
                            pan_ready += 1
            tile_base += m["ntiles"]
        # finalize branch stats -> stats[0] -> cc_in0
        for bi in range(8):
            vb1 = vmap[(bi, "b1")]
            nc.scalar.activation(out=fakeb[:, bi:bi + 1], in_=epsv[:, 0:1],
                                 func=AF.Identity, bias=vecs_t[:, vb1:vb1 + 1], scale=0.0)
        for bi in range(8):
            p0 = pan_col[bi]
            p1 = p0 + len(panels[bi])
            stat_reduce(tc, small, sums1, bn61, stats[0], 2 * bi, p0, p1, cnts1_t)
        nc.gpsimd.dma_start(out=cc_in[0][:], in_=stats[0][:])

    _emit_allreduce(nc, mybir, cc_in[0], cc_out[0], cc_sems[0])

    # =================== PHASE 2: bn1+relu -> z2 ===================
    with tile.TileContext(nc) as tc, ExitStack() as ctx:
        pool = ctx.enter_context(tc.tile_pool(name="p2", bufs=2))
        zps = ctx.enter_context(tc.tile_pool(name="zps2", bufs=3, space="PSUM"))
        entries = []
        for bi, (nm, dt, _, _) in enumerate(BRANCHES):
            nf = N_CORES * (RPAD[dt] - SHARD[dt])
            entries.append((2 * bi, NDIM[dt], vmap[(bi, "g1")], vmap[(bi, "bt1")],
                            fakeb[:, bi:bi + 1], nf))
        stats_to_scale(tc, pool, 0, entries)
        for bi, (nm, dt, _, _) in enumerate(BRANCHES):
            # fake col: h1 = relu(bn1(z1)); z2 = W2^T h1 + b2
            nc.scalar.activation(out=fakeb[:, bi:bi + 1], in_=fakeb[:, bi:bi + 1],
                                 func=AF.Relu, bias=sb[0][:, 2 * bi + 1:2 * bi + 2],
                                 scale=sb[0][:, 2 * bi:2 * bi + 1])
            zpf = zps.tile([128, 1], fp32, tag="zf")
            for half in (0, 1):
                hp = slice(half * D, (half + 1) * D)
                nc.tensor.matmul(out=zpf[hp, :], lhsT=wts_t[:, wmap[(bi, 2)], :][hp, :],
                                 rhs=fakeb[hp, bi:bi + 1], start=True, stop=True,
                                 tile_position=(half * 64, half * 64))
            vb2 = vmap[(bi, "b2")]
            nc.scalar.activation(out=fakeb[:, bi:bi + 1], in_=zpf[:, 0:1],
                                 func=AF.Identity, bias=vecs_t[:, vb2:vb2 + 1], scale=1.0)
        for bi, (nm, dt, _, _) in enumerate(BRANCHES):
            A = aoff[bi]
            for half in (0, 1):
                hp = slice(half * D, (half + 1) * D)
                for pi, (pc0, pw) in enumerate(panels[bi]):
                    # h1 = relu(scale*z1 + bias) in place
                    nc.scalar.activation(
                        out=acts[hp, A + pc0:A + pc0 + pw], in_=acts[hp, A + pc0:A + pc0 + pw],
                        func=AF.Relu, bias=sb[0][hp, 2 * bi + 1:2 * bi + 2],
                        scale=sb[0][hp, 2 * bi:2 * bi + 1])
                    zp = zps.tile([128, PANEL], fp32, tag="z2")
                    wv = wts_t[:, wmap[(bi, 2)], :]
                    nc.tensor.matmul(out=zp[hp, :pw], lhsT=wv[hp, :],
                                     rhs=acts[hp, A + pc0:A + pc0 + pw],
                                     start=True, stop=True,
                                     tile_position=(half * 64, half * 64))
                    j = pan_col[bi] + pi
                    vb2 = vmap[(bi, "b2")]
                    nc.scalar.activation(
                        out=acts[hp, A + pc0:A + pc0 + pw], in_=zp[hp, :pw],
                        func=AF.Identity, bias=vecs_t[hp, vb2:vb2 + 1], scale=1.0,
                        accum_out=sums2[hp, j:j + 1])
                    nc.vector.bn_stats(out=bn62[hp, 2 * j:2 * j + 2, :],
                                       in_=acts[hp, A + pc0:A + pc0 + pw])
        for bi in range(8):
            p0 = pan_col[bi]
            p1 = p0 + len(panels[bi])
            stat_reduce(tc, pool, sums2, bn62, stats[1], 2 * bi, p0, p1, cnts1_t)
        nc.gpsimd.dma_start(out=cc_in[1][:], in_=stats[1][:])

    _emit_allreduce(nc, mybir, cc_in[1], cc_out[1], cc_sems[1])

    # =================== PHASE 3: bn2+relu -> combine zc ===================
    with tile.TileContext(nc) as tc, ExitStack() as ctx:
        pool = ctx.enter_context(tc.tile_pool(name="p3", bufs=2))
        zps = ctx.enter_context(tc.tile_pool(name="zps3", bufs=3, space="PSUM"))
        entries = []
        for bi, (nm, dt, _, _) in enumerate(BRANCHES):
            nf = N_CORES * (RPAD[dt] - SHARD[dt])
            entries.append((2 * bi, NDIM[dt], vmap[(bi, "g2")], vmap[(bi, "bt2")],
                            fakeb[:, bi:bi + 1], nf))
        stats_to_scale(tc, pool, 1, entries)
        for d in DIMS:
            zpf = zps.tile([128, 1], fp32, tag="zfc")
            for k, bi in enumerate(COMB[d]):
                nc.scalar.activation(out=fakeb[:, bi:bi + 1], in_=fakeb[:, bi:bi + 1],
                                     func=AF.Relu, bias=sb[1][:, 2 * bi + 1:2 * bi + 2],
                                     scale=sb[1][:, 2 * bi:2 * bi + 1])
                for half in (0, 1):
                    hp = slice(half * D, (half + 1) * D)
                    nc.tensor.matmul(out=zpf[hp, :], lhsT=wts_t[:, wmap[("c", bi)], :][hp, :],
                                     rhs=fakeb[hp, bi:bi + 1], start=(k == 0),
                                     stop=(k == len(COMB[d]) - 1),
                                     tile_position=(half * 64, half * 64))
            vbc = vmap[(d, "bc")]
            nc.scalar.activation(out=fakec[:, d:d + 1], in_=zpf[:, 0:1],
                                 func=AF.Identity, bias=vecs_t[:, vbc:vbc + 1], scale=1.0)
        for d in DIMS:
            bis = COMB[d]
            A0 = aoff[bis[0]]
            for half in (0, 1):
                hp = slice(half * D, (half + 1) * D)
                for pi, (pc0, pw) in enumerate(panels[bis[0]]):
                    zp = zps.tile([128, PANEL], fp32, tag="zc")
                    for k, bi in enumerate(bis):
                        A = aoff[bi]
                        nc.scalar.activation(
                            out=acts[hp, A + pc0:A + pc0 + pw],
                            in_=acts[hp, A + pc0:A + pc0 + pw],
                            func=AF.Relu, bias=sb[1][hp, 2 * bi + 1:2 * bi + 2],
                            scale=sb[1][hp, 2 * bi:2 * bi + 1])
                        wv = wts_t[:, wmap[("c", bi)], :]
                        nc.tensor.matmul(out=zp[hp, :pw], lhsT=wv[hp, :],
                                         rhs=acts[hp, A + pc0:A + pc0 + pw],
                                         start=(k == 0), stop=(k == len(bis) - 1),
                                         tile_position=(half * 64, half * 64))
                    j = dpan_col[d] + pi
                    vbc = vmap[(d, "bc")]
                    nc.scalar.activation(
                        out=acts[hp, A0 + pc0:A0 + pc0 + pw], in_=zp[hp, :pw],
                        func=AF.Identity, bias=vecs_t[hp, vbc:vbc + 1], scale=1.0,
                        accum_out=sums3[hp, j:j + 1])
                    nc.vector.bn_stats(out=bn63[hp, 2 * j:2 * j + 2, :],
                                       in_=acts[hp, A0 + pc0:A0 + pc0 + pw])
        for d in DIMS:
            p0 = dpan_col[d]
            p1 = p0 + len(panels[COMB[d][0]])
            stat_reduce(tc, pool, sums3, bn63, stats[2], 2 * d, p0, p1, cnts3_t)
        nc.gpsimd.dma_start(out=cc_in[2][:], in_=stats[2][:])

    _emit_allreduce(nc, mybir, cc_in[2], cc_out[2], cc_sems[2])

    # =================== PHASE 4: bn3+relu -> out ===================
    with tile.TileContext(nc) as tc, ExitStack() as ctx:
        pool = ctx.enter_context(tc.tile_pool(name="p4", bufs=2))
        opool = ctx.enter_context(tc.tile_pool(name="o", bufs=3))
        entries = []
        for d in DIMS:
            nf = N_CORES * (RPAD[d] - SHARD[d])
            entries.append((2 * d, NDIM[d], vmap[(d, "gc")], vmap[(d, "btc")],
                            fakec[:, d:d + 1], nf))
        stats_to_scale(tc, pool, 2, entries)
        for d in DIMS:
            A0 = aoff[COMB[d][0]]
            for half in (0, 1):
                hp = slice(half * D, (half + 1) * D)
                for pi, (pc0, pw) in enumerate(panels[COMB[d][0]]):
                    o_t = opool.tile([128, PANEL], fp32, tag="o")
                    nc.scalar.activation(
                        out=o_t[hp, :pw], in_=acts[hp, A0 + pc0:A0 + pc0 + pw],
                        func=AF.Relu, bias=sb[2][hp, 2 * d + 1:2 * d + 2],
                        scale=sb[2][hp, 2 * d:2 * d + 1])
                    nc.sync.dma_start(out=out_e[d][half * D:(half + 1) * D, pc0:pc0 + pw],
                                      in_=o_t[hp, :pw])

    nc.compile()
    return nc, cnts1, cnts3


_LAST = None


def kernel(**inputs):
    from concourse.bass_utils import run_bass_kernel_spmd

    prep = _host_prep(inputs)
    nc, cnts1, cnts3 = _build_program(prep)

    in_maps = []
    for c in range(N_CORES):
        m = {"idx_all": prep["idx_all"][c], "rel_all": prep["rel_all"][c],
             "xt": prep["xt"][c], "wts": prep["wts"], "vecs": prep["vecs"],
             "iota": prep["iota"]}
        m.update(prep["core_tabs"][c])
        # cnts constants identical per core
        m["cnts1"] = cnts1
        m["cnts3"] = cnts3
        in_maps.append(m)

    global _LAST
    _LAST = (nc, in_maps)
    res = run_bass_kernel_spmd(nc, in_maps, list(range(N_CORES)))

    outs = {}
    for d in DIMS:
        S, H = SHARD[d], HALF[d]
        h = np.empty((NDIM[d], D), np.float32)
        for c in range(N_CORES):
            o = res.results[c][f"out{d}"]
            top = o[0:D, :].T           # rows [0,H)
            bot = o[D:2 * D, :].T       # rows [H, 2H)
            full = np.concatenate([top, bot], axis=0)[:S]
            h[c * S:(c + 1) * S] = full
        outs[d] = h
    return outs[0], outs[1], outs[2]


# revision 11
# speedup vs baseline: 2.0758x; 2.0758x over previous
"""DenseCINConv message-passing kernel for 8 Trainium2 NeuronCores.

Strategy: shard target cells (rows) of each dim across the 8 cores. Per
(branch, core): edges sorted by target block (128 targets), padded per block
to whole 128-edge tiles (tile counts equalized across cores so all cores run
one SPMD program). Sources are remapped to per-segment compacted fp16 tables
(<=32768 rows, int16 dma_gather indices; first-use order makes the gather
quasi-sequential in HBM). Aggregation = per-block selection-matrix matmuls
accumulated in PSUM, feature-major [64, targets]. Activations live in SBUF as
[128, rows/2] fp16 (features x two row-halves). MLP/BatchNorm run in 4 phases
separated by cross-core AllReduces of the BN statistics.
"""
import numpy as np

D = 64
P = 128
N_CORES = 8
BN_EPS = 1e-5
N0, N1, N2 = 100000, 200000, 75000
PANEL = 512
CALL_T = 24          # max tiles (128 edges) per dma_gather call
SEG_MAX = 32700      # max unique rows per compact table (int16 headroom)
XSLICE = 2048        # x_tgt stream slice (cols)

DIMS = [0, 1, 2]
NDIM = {0: N0, 1: N1, 2: N2}
SHARD = {d: NDIM[d] // N_CORES for d in DIMS}            # 12500, 25000, 9375
RPAD = {d: -(-SHARD[d] // (2 * P)) * (2 * P) for d in DIMS}   # 12544, 25088, 9472
HALF = {d: RPAD[d] // 2 for d in DIMS}                   # 6272, 12544, 4736

# branches: (name, tgt_dim, src_dim, index_key)
BRANCHES = [
    ("up0", 0, 0, "up0_index"),
    ("cob0", 0, 1, "cob0_index"),
    ("up1", 1, 1, "up1_index"),
    ("down1", 1, 1, "down1_index"),
    ("bnd1", 1, 0, "bnd1_index"),
    ("cob1", 1, 2, "cob1_index"),
    ("down2", 2, 2, "down2_index"),
    ("bnd2", 2, 1, "bnd2_index"),
]
# combine order per dim (branch indices into BRANCHES)
COMB = {0: [0, 1], 1: [2, 3, 4, 5], 2: [6, 7]}
# param sub-dict and key per branch
BPAR = {"up0": ("d0", "up"), "cob0": ("d0", "cob"), "up1": ("d1", "up"),
        "down1": ("d1", "down"), "bnd1": ("d1", "bnd"), "cob1": ("d1", "cob"),
        "down2": ("d2", "down"), "bnd2": ("d2", "bnd")}


def _first_use_codes(a):
    """Factorize int array by first-use order. Returns codes, uniques."""
    u, first, inv = np.unique(a, return_index=True, return_inverse=True)
    order = np.argsort(first, kind="stable")
    rank = np.empty(len(u), np.int64)
    rank[order] = np.arange(len(u))
    return rank[inv], u[order]


def _host_prep(inputs):
    """Build per-core gather streams / tables / metadata. Returns (meta, per_core_arrays)."""
    xw = {}
    for d, key in ((0, "x0"), (1, "x1"), (2, "x2")):
        x = np.asarray(inputs[key], np.float32)
        w = np.zeros((x.shape[0], 128), np.float16)
        w[:, :D] = x.astype(np.float16)
        xw[d] = w

    meta = []          # per (branch, half): dict with blocks' tile counts, segments, calls
    core_idx = [[] for _ in range(N_CORES)]    # int16 wrapped pieces
    core_rel = [[] for _ in range(N_CORES)]
    core_tabs = [{} for _ in range(N_CORES)]   # name -> array
    tab_shapes = {}

    for bi, (name, dt, ds, ikey) in enumerate(BRANCHES):
        idx = np.asarray(inputs[ikey])
        src_a, tgt_a = idx[0].astype(np.int64), idx[1].astype(np.int64)
        S, H = SHARD[dt], HALF[dt]
        core_of = tgt_a // S
        for half in (0, 1):
            nblk = H // P
            # per core, per block: src lists
            per_core_blocks = []
            for c in range(N_CORES):
                m = core_of == c
                tl = tgt_a[m] - c * S
                sb = src_a[m]
                hm = (tl >= H) == bool(half)
                r = tl[hm] - half * H
                s = sb[hm]
                blk = r // P
                rel = (r % P).astype(np.float32)
                order = np.argsort(blk, kind="stable")
                blk, rel, s = blk[order], rel[order], s[order]
                bounds = np.searchsorted(blk, np.arange(nblk + 1))
                per_core_blocks.append((s, rel, bounds))
            tiles_per_blk = np.zeros(nblk, np.int64)
            for c in range(N_CORES):
                _, _, bounds = per_core_blocks[c]
                cnt = bounds[1:] - bounds[:-1]
                tiles_per_blk = np.maximum(tiles_per_blk, -(-cnt // P))
            tiles_per_blk = np.maximum(tiles_per_blk, 1)

            # build padded streams per core (src full-ids, rel with -1 pads)
            streams = []
            for c in range(N_CORES):
                s, rel, bounds = per_core_blocks[c]
                ss, rr = [], []
                for b in range(nblk):
                    sb = s[bounds[b]:bounds[b + 1]]
                    rb = rel[bounds[b]:bounds[b + 1]]
                    pad = tiles_per_blk[b] * P - len(sb)
                    ss.append(sb); ss.append(np.full(pad, -1, np.int64))
                    rr.append(rb); rr.append(np.full(pad, -1.0, np.float32))
                streams.append((np.concatenate(ss), np.concatenate(rr)))

            # segment blocks so per-segment uniques (plus zero row) <= SEG_MAX for every core
            seg_bounds = [0]
            sets = [set() for _ in range(N_CORES)]
            for b in range(nblk):
                trial = []
                for c in range(N_CORES):
                    s, _, bounds = per_core_blocks[c]
                    trial.append(set(s[bounds[b]:bounds[b + 1]].tolist()))
                if any(len(sets[c] | trial[c]) + 1 > SEG_MAX for c in range(N_CORES)):
                    seg_bounds.append(b)
                    sets = trial
                else:
                    for c in range(N_CORES):
                        sets[c] |= trial[c]
            seg_bounds.append(nblk)

            tile_off = np.concatenate([[0], np.cumsum(tiles_per_blk)])
            segs = []
            for si in range(len(seg_bounds) - 1):
                b0, b1 = seg_bounds[si], seg_bounds[si + 1]
                t0, t1 = int(tile_off[b0]), int(tile_off[b1])
                tabname = f"tab_{bi}_{half}_{si}"
                umax = 0
                codes_per_core = []
                for c in range(N_CORES):
                    s_pad, _ = streams[c]
                    seg_src = s_pad[t0 * P:t1 * P]
                    realm = seg_src >= 0
                    codes = np.zeros(len(seg_src), np.int64)
                    if realm.any():
                        cds, uniq = _first_use_codes(seg_src[realm])
                        codes[realm] = cds + 1
                        tab = np.zeros((len(uniq) + 1, 128), np.float16)
                        tab[1:] = xw[ds][uniq]
                    else:
                        tab = np.zeros((1, 128), np.float16)
                    umax = max(umax, tab.shape[0])
                    codes_per_core.append(codes)
                    core_tabs[c][tabname] = tab
                for c in range(N_CORES):
                    t = core_tabs[c][tabname]
                    if t.shape[0] < umax:
                        core_tabs[c][tabname] = np.vstack(
                            [t, np.zeros((umax - t.shape[0], 128), np.float16)])
                tab_shapes[tabname] = umax
                # calls: group whole blocks, <= CALL_T tiles each
                calls = []
                cb0 = b0
                while cb0 < b1:
                    cb1 = cb0
                    tsum = 0
                    while cb1 < b1 and tsum + tiles_per_blk[cb1] <= CALL_T:
                        tsum += int(tiles_per_blk[cb1]); cb1 += 1
                    if cb1 == cb0:
                        cb1 = cb0 + 1
                        tsum = int(tiles_per_blk[cb0])
                    calls.append((int(tile_off[cb0]), tsum,
                                  [(int(b), int(tiles_per_blk[b])) for b in range(cb0, cb1)]))
                    cb0 = cb1
                segs.append({"name": tabname, "t0": t0, "t1": t1, "calls": calls,
                             "codes": codes_per_core})
            # append per-core wrapped idx / rel
            for c in range(N_CORES):
                s_pad, r_pad = streams[c]
                ntile = len(s_pad) // P
                codes_all = np.zeros(len(s_pad), np.int16)
                for sg in segs:
                    codes_all[sg["t0"] * P:sg["t1"] * P] = \
                        sg["codes"][c].astype(np.int16)
                wrapped = codes_all.reshape(-1, 16).T      # [16, L/16]
                core_idx[c].append(np.tile(wrapped, (8, 1)))
                core_rel[c].append(r_pad.astype(np.float16).reshape(ntile, P).T)
            for sg in segs:
                del sg["codes"]
            meta.append({"bi": bi, "dt": dt, "half": half, "nblk": nblk,
                         "tiles_per_blk": tiles_per_blk.tolist(), "segs": segs,
                         "ntiles": int(tile_off[-1])})

    idx_all = [np.concatenate(core_idx[c], axis=1) for c in range(N_CORES)]
    rel_all = [np.concatenate(core_rel[c], axis=1) for c in range(N_CORES)]

    # x_tgt device layout [128, H0+H1+H2] fp16 (per dim: top rows feats 0-63, bottom 64-127)
    xoff = {}
    off = 0
    for d in DIMS:
        xoff[d] = off
        off += HALF[d]
    xt = [np.zeros((128, off), np.float16) for _ in range(N_CORES)]
    for c in range(N_CORES):
        for d, key in ((0, "x0"), (1, "x1"), (2, "x2")):
            x = np.asarray(inputs[key], np.float32).astype(np.float16)
            S, H, R = SHARD[d], HALF[d], RPAD[d]
            shard = np.zeros((R, D), np.float16)
            shard[:S] = x[c * S:(c + 1) * S]
            xt[c][0:D, xoff[d]:xoff[d] + H] = shard[:H].T
            xt[c][D:2 * D, xoff[d]:xoff[d] + H] = shard[H:].T

    # weights / vectors
    params = inputs["params"]
    wts, vecs = [], []
    wmap, vmap = {}, {}

    def addw(w):
        wts.append(np.vstack([w, w]).astype(np.float16))
        return len(wts) - 1

    def addv(v):
        vecs.append(np.concatenate([v, v]).astype(np.float32))
        return len(vecs) - 1

    for bi, (name, dt, ds, _) in enumerate(BRANCHES):
        dk, bk = BPAR[name]
        p = {k: np.asarray(v, np.float32) for k, v in params[dk][bk].items()}
        wmap[(bi, 1)] = addw(p["W1"])
        wmap[(bi, 2)] = addw(p["W2"])
        for k in ("b1", "g1", "bt1", "b2", "g2", "bt2"):
            vmap[(bi, k)] = addv(p[k])
    for d in DIMS:
        pc = {k: np.asarray(v, np.float32) for k, v in params[f"d{d}"]["comb"].items()}
        for j, bi in enumerate(COMB[d]):
            wmap[("c", bi)] = addw(pc["Wc"][j * D:(j + 1) * D])
        for k in ("bc", "gc", "btc"):
            vmap[(d, k)] = addv(pc[k])
    wts_arr = np.stack(wts).transpose(1, 0, 2).copy()  # [128, NW, 64] fp16
    vecs_arr = np.stack(vecs).reshape(len(vecs), 128).T.copy()  # [128, NV] fp32

    iota = np.tile(np.arange(P, dtype=np.float16), (P, 1))

    return {
        "meta": meta, "tab_shapes": tab_shapes, "xoff": xoff,
        "wts": wts_arr, "vecs": vecs_arr, "wmap": wmap, "vmap": vmap,
        "iota": iota, "idx_all": idx_all, "rel_all": rel_all, "xt": xt,
        "core_tabs": core_tabs,
    }


def _emit_allreduce(nc, mybir, cin, cout, sem):
    with nc.Block() as block:
        @block.gpsimd
        def _(g):
            g.collective_compute(
                "AllReduce", mybir.AluOpType.add,
                replica_groups=[list(range(N_CORES))],
                ins=[cin[:]], outs=[cout[:]]).then_inc(sem, 1)
            g.wait_ge(sem, 1)


def _build_program(prep):
    import os
    SKIP_GATHER = os.environ.get("SKIP_GATHER") == "1"
    SKIP_AGG = os.environ.get("SKIP_AGG") == "1"
    import concourse.bacc as bacc
    import concourse.tile as tile
    from concourse import mybir
    from contextlib import ExitStack

    fp16, fp32, i16 = mybir.dt.float16, mybir.dt.float32, mybir.dt.int16
    AF = mybir.ActivationFunctionType
    OP = mybir.AluOpType

    meta = prep["meta"]
    TOT_TILES = sum(m["ntiles"] for m in meta)
    NW = prep["wts"].shape[1]
    NV = prep["vecs"].shape[1]
    XTOT = sum(HALF[d] for d in DIMS)
    xoff = prep["xoff"]

    nc = bacc.Bacc("TRN2", target_bir_lowering=False)

    # ---- DRAM params
    tabs_e = {n: nc.declare_dram_parameter(n, [u, 128], fp16, isOutput=False)
              for n, u in prep["tab_shapes"].items()}
    idx_e = nc.declare_dram_parameter("idx_all", [128, TOT_TILES * 8], i16, isOutput=False)
    rel_e = nc.declare_dram_parameter("rel_all", [128, TOT_TILES], fp16, isOutput=False)
    xt_e = nc.declare_dram_parameter("xt", [128, XTOT], fp16, isOutput=False)
    wts_e = nc.declare_dram_parameter("wts", [128, NW, D], fp16, isOutput=False)
    vecs_e = nc.declare_dram_parameter("vecs", [128, NV], fp32, isOutput=False)
    iota_e = nc.declare_dram_parameter("iota", [P, P], fp16, isOutput=False)
    out_e = {d: nc.declare_dram_parameter(f"out{d}", [128, HALF[d]], fp32, isOutput=True)
             for d in DIMS}

    cc_in = [nc.dram_tensor(f"cc_in{k}", [128, 16], fp32) for k in range(3)]
    cc_out = [nc.dram_tensor(f"cc_out{k}", [128, 16], fp32, addr_space="Shared")
              for k in range(3)]
    comb_dram = [nc.dram_tensor(f"comb{k}", [D, 16], fp32) for k in range(3)]

    # ---- persistent SBUF
    # acts: per branch a region of HALF[dt] cols
    aoff = {}
    off = 0
    for bi, (nm, dt, _, _) in enumerate(BRANCHES):
        aoff[bi] = off
        off += HALF[dt]
    ATOT = off
    acts = nc.alloc_sbuf_tensor("acts", [128, ATOT], fp16)
    rel_t = nc.alloc_sbuf_tensor("rel_t", [128, TOT_TILES], fp16)
    iota_t = nc.alloc_sbuf_tensor("iota_t", [P, P], fp16)
    wts_t = nc.alloc_sbuf_tensor("wts_t", [128, NW, D], fp16)
    vecs_t = nc.alloc_sbuf_tensor("vecs_t", [128, NV], fp32)

    # panels per branch: list of (col0, width); panel grid shared by layers
    panels = {}
    NPAN = 0
    pan_col = {}      # (bi) -> start index into stat columns
    for bi, (nm, dt, _, _) in enumerate(BRANCHES):
        H = HALF[dt]
        ps = []
        c0 = 0
        while c0 < H:
            w = min(PANEL, H - c0)
            ps.append((c0, w))
            c0 += w
        panels[bi] = ps
        pan_col[bi] = NPAN
        NPAN += len(ps)
    # combine (dim) panels use branch0 regions
    dpan_col = {}
    NPANC = 0
    for d in DIMS:
        dpan_col[d] = NPANC
        NPANC += len(panels[COMB[d][0]])

    sums1 = nc.alloc_sbuf_tensor("sums1", [128, NPAN], fp32)
    bn61 = nc.alloc_sbuf_tensor("bn61", [128, 2 * NPAN, 3], fp32)
    sums2 = nc.alloc_sbuf_tensor("sums2", [128, NPAN], fp32)
    bn62 = nc.alloc_sbuf_tensor("bn62", [128, 2 * NPAN, 3], fp32)
    sums3 = nc.alloc_sbuf_tensor("sums3", [128, NPANC], fp32)
    bn63 = nc.alloc_sbuf_tensor("bn63", [128, 2 * NPANC, 3], fp32)
    stats = [nc.alloc_sbuf_tensor(f"stats{k}", [128, 16], fp32) for k in range(3)]
    epsv = nc.alloc_sbuf_tensor("epsv", [128, 1], fp32)
    fakeb = nc.alloc_sbuf_tensor("fakeb", [128, 8], fp16)   # per-branch fake activation
    fakec = nc.alloc_sbuf_tensor("fakec", [128, 3], fp16)   # per-dim fake zc
    sb = [nc.alloc_sbuf_tensor(f"sb{k}", [128, 16], fp32) for k in range(3)]

    cc_sems = [nc.alloc_semaphore(f"ccs{k}") for k in range(3)]

    def stat_reduce(tc, pool, sums_t, bn6_t, stats_t, col, p0, p1, cnts_t):
        """reduce per-panel stats [p0,p1) -> stats_t[:, col(2)] = (sum, sumsq)."""
        nc.vector.tensor_reduce(out=stats_t[:, col:col + 1], in_=sums_t[:, p0:p1],
                                axis=mybir.AxisListType.X, op=OP.add)
        musq = pool.tile([128, 2 * (p1 - p0)], fp32, tag="musq")
        nc.vector.tensor_tensor(out=musq[:], in0=bn6_t[:, 2 * p0:2 * p1, 1],
                                in1=bn6_t[:, 2 * p0:2 * p1, 1], op=OP.mult)
        nc.vector.tensor_tensor(out=musq[:], in0=musq[:],
                                in1=cnts_t[:, 2 * p0:2 * p1], op=OP.mult)
        t2 = pool.tile([128, 2], fp32, tag="sqp")
        nc.vector.tensor_reduce(out=t2[:, 0:1], in_=bn6_t[:, 2 * p0:2 * p1, 2],
                                axis=mybir.AxisListType.X, op=OP.add)
        nc.vector.tensor_reduce(out=t2[:, 1:2], in_=musq[:],
                                axis=mybir.AxisListType.X, op=OP.add)
        nc.vector.tensor_tensor(out=stats_t[:, col + 1:col + 2], in0=t2[:, 0:1],
                                in1=t2[:, 1:2], op=OP.add)

    def stats_to_scale(tc, pool, k, entries):
        """After allreduce k: load cc_out, combine halves, compute scale/bias into sb[k].

        entries: (col, Ntrue, g_vec_idx, bt_vec_idx, fake_ap, nfake); fake_ap is a
        [128,1] fp16 column holding the layer value fake (padded) rows carry; its
        contribution is removed from the all-reduced sums before mean/var."""
        top = pool.tile([D, 16], fp32, tag="cctop")
        bot = pool.tile([D, 16], fp32, tag="ccbot")
        nc.gpsimd.dma_start(out=top[:], in_=cc_out[k][0:D, :])
        nc.gpsimd.dma_start(out=bot[:], in_=cc_out[k][D:2 * D, :])
        nc.vector.tensor_tensor(out=top[:], in0=top[:], in1=bot[:], op=OP.add)
        nc.gpsimd.dma_start(out=comb_dram[k][:], in_=top[:])
        nc.gpsimd.dma_start(out=stats[k][0:D, :], in_=comb_dram[k][:])
        nc.gpsimd.dma_start(out=stats[k][D:2 * D, :], in_=comb_dram[k][:])
        for (col, Ntrue, gi, bti, fake_ap, nfake) in entries:
            mean = pool.tile([128, 1], fp32, tag="mean")
            e2 = pool.tile([128, 1], fp32, tag="e2")
            if nfake:
                sumc = pool.tile([128, 1], fp32, tag="sumc")
                nc.vector.scalar_tensor_tensor(
                    out=sumc[:], in0=fake_ap, scalar=-float(nfake),
                    in1=stats[k][:, col:col + 1], op0=OP.mult, op1=OP.add)
                fsq = pool.tile([128, 1], fp32, tag="fsq")
                nc.vector.tensor_tensor(out=fsq[:], in0=fake_ap, in1=fake_ap, op=OP.mult)
                sqc = pool.tile([128, 1], fp32, tag="sqc")
                nc.vector.scalar_tensor_tensor(
                    out=sqc[:], in0=fsq[:], scalar=-float(nfake),
                    in1=stats[k][:, col + 1:col + 2], op0=OP.mult, op1=OP.add)
                nc.vector.tensor_scalar_mul(mean[:], sumc[:], 1.0 / Ntrue)
                nc.vector.tensor_scalar_mul(e2[:], sqc[:], 1.0 / Ntrue)
            else:
                nc.vector.tensor_scalar_mul(mean[:], stats[k][:, col:col + 1], 1.0 / Ntrue)
                nc.vector.tensor_scalar_mul(e2[:], stats[k][:, col + 1:col + 2], 1.0 / Ntrue)
            var = pool.tile([128, 1], fp32, tag="var")
            nc.vector.tensor_tensor(out=var[:], in0=mean[:], in1=mean[:], op=OP.mult)
            nc.vector.tensor_tensor(out=var[:], in0=e2[:], in1=var[:], op=OP.subtract)
            nc.vector.tensor_scalar_max(var[:], var[:], 0.0)
            sig = pool.tile([128, 1], fp32, tag="sig")
            nc.scalar.activation(out=sig[:], in_=var[:], func=AF.Sqrt, bias=epsv[:, 0:1], scale=1.0)
            inv = pool.tile([128, 1], fp32, tag="inv")
            nc.vector.reciprocal(out=inv[:], in_=sig[:])
            nc.vector.tensor_tensor(out=sb[k][:, col:col + 1], in0=vecs_t[:, gi:gi + 1],
                                    in1=inv[:], op=OP.mult)
            t = pool.tile([128, 1], fp32, tag="tmp")
            nc.vector.tensor_tensor(out=t[:], in0=mean[:], in1=sb[k][:, col:col + 1],
                                    op=OP.mult)
            nc.vector.tensor_tensor(out=sb[k][:, col + 1:col + 2],
                                    in0=vecs_t[:, bti:bti + 1], in1=t[:], op=OP.subtract)

    wmap, vmap = prep["wmap"], prep["vmap"]

    # cnts constants for stat aggregation (per panel: count of even/odd elems per bn_stats)
    cnts1 = np.zeros((128, 2 * NPAN), np.float32)
    for bi in range(8):
        for pi, (c0, w) in enumerate(panels[bi]):
            j = pan_col[bi] + pi
            cnts1[:, 2 * j] = (w + 1) // 2
            cnts1[:, 2 * j + 1] = w // 2
    cnts3 = np.zeros((128, 2 * NPANC), np.float32)
    for d in DIMS:
        for pi, (c0, w) in enumerate(panels[COMB[d][0]]):
            j = dpan_col[d] + pi
            cnts3[:, 2 * j] = (w + 1) // 2
            cnts3[:, 2 * j + 1] = w // 2
    cnts1_e = nc.declare_dram_parameter("cnts1", [128, 2 * NPAN], fp32, isOutput=False)
    cnts3_e = nc.declare_dram_parameter("cnts3", [128, 2 * NPANC], fp32, isOutput=False)
    cnts1_t = nc.alloc_sbuf_tensor("cnts1_t", [128, 2 * NPAN], fp32)
    cnts3_t = nc.alloc_sbuf_tensor("cnts3_t", [128, 2 * NPANC], fp32)

    # =================== PHASE 1 ===================
    with tile.TileContext(nc) as tc, ExitStack() as ctx:
        gpool = ctx.enter_context(tc.tile_pool(name="g", bufs=3))
        ipool = ctx.enter_context(tc.tile_pool(name="i", bufs=3))
        spool = ctx.enter_context(tc.tile_pool(name="s", bufs=6))
        xpool = ctx.enter_context(tc.tile_pool(name="x", bufs=2))
        scpool = ctx.enter_context(tc.tile_pool(name="sc", bufs=2))
        aps = ctx.enter_context(tc.tile_pool(name="aps", bufs=4, space="PSUM"))
        zps = ctx.enter_context(tc.tile_pool(name="zps", bufs=2, space="PSUM"))
        small = ctx.enter_context(tc.tile_pool(name="sm", bufs=2))

        nc.sync.dma_start(out=rel_t[:], in_=rel_e[:])
        nc.sync.dma_start(out=iota_t[:], in_=iota_e[:])
        nc.sync.dma_start(out=wts_t[:], in_=wts_e[:])
        nc.sync.dma_start(out=vecs_t[:], in_=vecs_e[:])
        nc.sync.dma_start(out=cnts1_t[:], in_=cnts1_e[:])
        nc.sync.dma_start(out=cnts3_t[:], in_=cnts3_e[:])
        nc.vector.memset(epsv[:], BN_EPS)

        tile_base = 0
        for m in meta:
            bi, dt, half = m["bi"], m["dt"], m["half"]
            H = HALF[dt]
            hp = slice(half * D, (half + 1) * D)
            tp = (0, 0) if half == 0 else (0, 64)
            A = aoff[bi]
            tpb = m["tiles_per_blk"]
            # x_tgt slices for this (dim, half): xt[hp, xoff+.. ]
            blk_done = 0
            pan_ready = 0
            xt_tile = None
            xt_lo = -1
            psum_by_blk = {}
            for sg in m["segs"]:
                tab = tabs_e[sg["name"]]
                for (t0, ntiles, blocks) in sg["calls"]:
                    idx_t = ipool.tile([128, CALL_T * 8], i16, tag="idx")
                    nc.sync.dma_start(out=idx_t[:, :ntiles * 8],
                                      in_=idx_e[:, (tile_base + t0) * 8:(tile_base + t0 + ntiles) * 8])
                    g_t = gpool.tile([128, CALL_T, 128], fp16, tag="g")
                    if SKIP_GATHER:
                        nc.vector.memset(g_t[:, :ntiles, 0:D], 0)
                    else:
                        nc.gpsimd.dma_gather(
                            out_ap=g_t[:, :ntiles, :], in_ap=tab[:],
                            idxs_ap=idx_t[:, :ntiles * 8],
                            num_idxs=ntiles * P, num_idxs_reg=ntiles * P,
                            elem_size=128, single_packet=False)
                    kloc = 0
                    for (b, tb) in blocks:
                        ps = aps.tile([128, P], fp32, tag="aggps")
                        if SKIP_AGG:
                            nc.vector.memset(ps[hp, :], 0)
                        for k in range(0 if SKIP_AGG else tb):
                            gcol = tile_base + t0 + kloc + k
                            s_t = spool.tile([P, P], fp16, tag="s")
                            nc.vector.tensor_tensor(
                                out=s_t[:],
                                in0=rel_t[:, gcol:gcol + 1].to_broadcast([P, P]),
                                in1=iota_t[:], op=OP.is_equal)
                            nc.tensor.matmul(out=ps[hp, :], lhsT=g_t[:, kloc + k, 0:D],
                                             rhs=s_t[:], start=(k == 0), stop=(k == tb - 1),
                                             tile_position=tp)
                        kloc += tb
                        # X = agg + x_tgt
                        c0 = b * P
                        if xt_lo < 0 or c0 >= xt_lo + XSLICE:
                            xt_lo = (c0 // XSLICE) * XSLICE
                            w = min(XSLICE, H - xt_lo)
                            xt_tile = xpool.tile([128, XSLICE], fp16, tag="xt")
                            nc.sync.dma_start(out=xt_tile[:, :w],
                                              in_=xt_e[:, xoff[dt] + xt_lo:xoff[dt] + xt_lo + w])
                        nc.vector.tensor_tensor(
                            out=acts[hp, A + c0:A + c0 + P], in0=ps[hp, :],
                            in1=xt_tile[hp, c0 - xt_lo:c0 - xt_lo + P], op=OP.add)
                        blk_done += 1
                        # z1 for completed panels
                        while pan_ready < len(panels[bi]):
                            pc0, pw = panels[bi][pan_ready]
                            if blk_done * P < pc0 + pw:
                                break
                            zp = zps.tile([128, PANEL], fp32, tag="z")
                            wv = wts_t[:, wmap[(bi, 1)], :]
                            nc.tensor.matmul(out=zp[hp, :pw], lhsT=wv[hp, :],
                                             rhs=acts[hp, A + pc0:A + pc0 + pw],
                                             start=True, stop=True,
                                             tile_position=(half * 64, half * 64))
                            j = pan_col[bi] + pan_ready
                            vb1 = vmap[(bi, "b1")]
                            nc.scalar.activation(
                                out=acts[hp, A + pc0:A + pc0 + pw], in_=zp[hp, :pw],
                                func=AF.Identity, bias=vecs_t[hp, vb1:vb1 + 1], scale=1.0,
                                accum_out=sums1[hp, j:j + 1])
                            nc.vector.bn_stats(out=bn61[hp, 2 * j:2 * j + 2, :],
                                               in_=acts[hp, A + pc0:A + pc0 + pw])
                            pan_ready += 1
            tile_base += m["ntiles"]
        # finalize branch stats -> stats[0] -> cc_in0
        for bi in range(8):
            vb1 = vmap[(bi, "b1")]
            nc.scalar.activation(out=fakeb[:, bi:bi + 1], in_=epsv[:, 0:1],
                                 func=AF.Identity, bias=vecs_t[:, vb1:vb1 + 1], scale=0.0)
        for bi in range(8):
            p0 = pan_col[bi]
            p1 = p0 + len(panels[bi])
            stat_reduce(tc, small, sums1, bn61, stats[0], 2 * bi, p0, p1, cnts1_t)
        nc.gpsimd.dma_start(out=cc_in[0][:], in_=stats[0][:])

    _emit_allreduce(nc, mybir, cc_in[0], cc_out[0], cc_sems[0])

    # =================== PHASE 2: bn1+relu -> z2 ===================
    with tile.TileContext(nc) as tc, ExitStack() as ctx:
        pool = ctx.enter_context(tc.tile_pool(name="p2", bufs=2))
        zps = ctx.enter_context(tc.tile_pool(name="zps2", bufs=3, space="PSUM"))
        entries = []
        for bi, (nm, dt, _, _) in enumerate(BRANCHES):
            nf = N_CORES * (RPAD[dt] - SHARD[dt])
            entries.append((2 * bi, NDIM[dt], vmap[(bi, "g1")], vmap[(bi, "bt1")],
                            fakeb[:, bi:bi + 1], nf))
        stats_to_scale(tc, pool, 0, entries)
        for bi, (nm, dt, _, _) in enumerate(BRANCHES):
            # fake col: h1 = relu(bn1(z1)); z2 = W2^T h1 + b2
            nc.scalar.activation(out=fakeb[:, bi:bi + 1], in_=fakeb[:, bi:bi + 1],
                                 func=AF.Relu, bias=sb[0][:, 2 * bi + 1:2 * bi + 2],
                                 scale=sb[0][:, 2 * bi:2 * bi + 1])
            zpf = zps.tile([128, 1], fp32, tag="zf")
            for half in (0, 1):
                hp = slice(half * D, (half + 1) * D)
                nc.tensor.matmul(out=zpf[hp, :], lhsT=wts_t[:, wmap[(bi, 2)], :][hp, :],
                                 rhs=fakeb[hp, bi:bi + 1], start=True, stop=True,
                                 tile_position=(half * 64, half * 64))
            vb2 = vmap[(bi, "b2")]
            nc.scalar.activation(out=fakeb[:, bi:bi + 1], in_=zpf[:, 0:1],
                                 func=AF.Identity, bias=vecs_t[:, vb2:vb2 + 1], scale=1.0)
        for bi, (nm, dt, _, _) in enumerate(BRANCHES):
            A = aoff[bi]
            for half in (0, 1):
                hp = slice(half * D, (half + 1) * D)
                for pi, (pc0, pw) in enumerate(panels[bi]):
                    # h1 = relu(scale*z1 + bias) in place
                    nc.scalar.activation(
                        out=acts[hp, A + pc0:A + pc0 + pw], in_=acts[hp, A + pc0:A + pc0 + pw],
                        func=AF.Relu, bias=sb[0][hp, 2 * bi + 1:2 * bi + 2],
                        scale=sb[0][hp, 2 * bi:2 * bi + 1])
                    zp = zps.tile([128, PANEL], fp32, tag="z2")
                    wv = wts_t[:, wmap[(bi, 2)], :]
                    nc.tensor.matmul(out=zp[hp, :pw], lhsT=wv[hp, :],
                                     rhs=acts[hp, A + pc0:A + pc0 + pw],
                                     start=True, stop=True,
                                     tile_position=(half * 64, half * 64))
                    j = pan_col[bi] + pi
                    vb2 = vmap[(bi, "b2")]
                    nc.scalar.activation(
                        out=acts[hp, A + pc0:A + pc0 + pw], in_=zp[hp, :pw],
                        func=AF.Identity, bias=vecs_t[hp, vb2:vb2 + 1], scale=1.0,
                        accum_out=sums2[hp, j:j + 1])
                    nc.vector.bn_stats(out=bn62[hp, 2 * j:2 * j + 2, :],
                                       in_=acts[hp, A + pc0:A + pc0 + pw])
        for bi in range(8):
            p0 = pan_col[bi]
            p1 = p0 + len(panels[bi])
            stat_reduce(tc, pool, sums2, bn62, stats[1], 2 * bi, p0, p1, cnts1_t)
        nc.gpsimd.dma_start(out=cc_in[1][:], in_=stats[1][:])

    _emit_allreduce(nc, mybir, cc_in[1], cc_out[1], cc_sems[1])

    # =================== PHASE 3: bn2+relu -> combine zc ===================
    with tile.TileContext(nc) as tc, ExitStack() as ctx:
        pool = ctx.enter_context(tc.tile_pool(name="p3", bufs=2))
        zps = ctx.enter_context(tc.tile_pool(name="zps3", bufs=3, space="PSUM"))
        entries = []
        for bi, (nm, dt, _, _) in enumerate(BRANCHES):
            nf = N_CORES * (RPAD[dt] - SHARD[dt])
            entries.append((2 * bi, NDIM[dt], vmap[(bi, "g2")], vmap[(bi, "bt2")],
                            fakeb[:, bi:bi + 1], nf))
        stats_to_scale(tc, pool, 1, entries)
        for d in DIMS:
            zpf = zps.tile([128, 1], fp32, tag="zfc")
            for k, bi in enumerate(COMB[d]):
                nc.scalar.activation(out=fakeb[:, bi:bi + 1], in_=fakeb[:, bi:bi + 1],
                                     func=AF.Relu, bias=sb[1][:, 2 * bi + 1:2 * bi + 2],
                                     scale=sb[1][:, 2 * bi:2 * bi + 1])
                for half in (0, 1):
                    hp = slice(half * D, (half + 1) * D)
                    nc.tensor.matmul(out=zpf[hp, :], lhsT=wts_t[:, wmap[("c", bi)], :][hp, :],
                                     rhs=fakeb[hp, bi:bi + 1], start=(k == 0),
                                     stop=(k == len(COMB[d]) - 1),
                                     tile_position=(half * 64, half * 64))
            vbc = vmap[(d, "bc")]
            nc.scalar.activation(out=fakec[:, d:d + 1], in_=zpf[:, 0:1],
                                 func=AF.Identity, bias=vecs_t[:, vbc:vbc + 1], scale=1.0)
        for d in DIMS:
            bis = COMB[d]
            A0 = aoff[bis[0]]
            for half in (0, 1):
                hp = slice(half * D, (half + 1) * D)
                for pi, (pc0, pw) in enumerate(panels[bis[0]]):
                    zp = zps.tile([128, PANEL], fp32, tag="zc")
                    for k, bi in enumerate(bis):
                        A = aoff[bi]
                        nc.scalar.activation(
                            out=acts[hp, A + pc0:A + pc0 + pw],
                            in_=acts[hp, A + pc0:A + pc0 + pw],
                            func=AF.Relu, bias=sb[1][hp, 2 * bi + 1:2 * bi + 2],
                            scale=sb[1][hp, 2 * bi:2 * bi + 1])
                        wv = wts_t[:, wmap[("c", bi)], :]
                        nc.tensor.matmul(out=zp[hp, :pw], lhsT=wv[hp, :],
                                         rhs=acts[hp, A + pc0:A + pc0 + pw],
                                         start=(k == 0), stop=(k == len(bis) - 1),
                                         tile_position=(half * 64, half * 64))
                    j = dpan_col[d] + pi
                    vbc = vmap[(d, "bc")]
                    nc.scalar.activation(
                        out=acts[hp, A0 + pc0:A0 + pc0 + pw], in_=zp[hp, :pw],
                        func=AF.Identity, bias=vecs_t[hp, vbc:vbc + 1], scale=1.0,
                        accum_out=sums3[hp, j:j + 1])
                    nc.vector.bn_stats(out=bn63[hp, 2 * j:2 * j + 2, :],
                                       in_=acts[hp, A0 + pc0:A0 + pc0 + pw])
        for d in DIMS:
            p0 = dpan_col[d]
            p1 = p0 + len(panels[COMB[d][0]])
            stat_reduce(tc, pool, sums3, bn63, stats[2], 2 * d, p0, p1, cnts3_t)
        nc.gpsimd.dma_start(out=cc_in[2][:], in_=stats[2][:])

    _emit_allreduce(nc, mybir, cc_in[2], cc_out[2], cc_sems[2])

    # =================== PHASE 4: bn3+relu -> out ===================
    with tile.TileContext(nc) as tc, ExitStack() as ctx:
        pool = ctx.enter_context(tc.tile_pool(name="p4", bufs=2))
        opool = ctx.enter_context(tc.tile_pool(name="o", bufs=3))
        entries = []
        for d in DIMS:
            nf = N_CORES * (RPAD[d] - SHARD[d])
            entries.append((2 * d, NDIM[d], vmap[(d, "gc")], vmap[(d, "btc")],
                            fakec[:, d:d + 1], nf))
        stats_to_scale(tc, pool, 2, entries)
        for d in DIMS:
            A0 = aoff[COMB[d][0]]
            for half in (0, 1):
                hp = slice(half * D, (half + 1) * D)
                for pi, (pc0, pw) in enumerate(panels[COMB[d][0]]):
                    o_t = opool.tile([128, PANEL], fp32, tag="o")
                    nc.scalar.activation(
                        out=o_t[hp, :pw], in_=acts[hp, A0 + pc0:A0 + pc0 + pw],
                        func=AF.Relu, bias=sb[2][hp, 2 * d + 1:2 * d + 2],
                        scale=sb[2][hp, 2 * d:2 * d + 1])
                    nc.sync.dma_start(out=out_e[d][half * D:(half + 1) * D, pc0:pc0 + pw],
                                      in_=o_t[hp, :pw])

    nc.compile()
    return nc, cnts1, cnts3


_LAST = None


def kernel(**inputs):
    from concourse.bass_utils import run_bass_kernel_spmd

    prep = _host_prep(inputs)
    nc, cnts1, cnts3 = _build_program(prep)

    in_maps = []
    for c in range(N_CORES):
        m = {"idx_all": prep["idx_all"][c], "rel_all": prep["rel_all"][c],
             "xt": prep["xt"][c], "wts": prep["wts"], "vecs": prep["vecs"],
             "iota": prep["iota"]}
        m.update(prep["core_tabs"][c])
        # cnts constants identical per core
        m["cnts1"] = cnts1
        m["cnts3"] = cnts3
        in_maps.append(m)

    global _LAST
    _LAST = (nc, in_maps)
    res = run_bass_kernel_spmd(nc, in_maps, list(range(N_CORES)))

    outs = {}
    for d in DIMS:
        S, H = SHARD[d], HALF[d]
        h = np.empty((NDIM[d], D), np.float32)
        for c in range(N_CORES):
            o = res.results[c][f"out{d}"]
            top = o[0:D, :].T           # rows [0,H)
            bot = o[D:2 * D, :].T       # rows [H, 2H)
            full = np.concatenate([top, bot], axis=0)[:S]
            h[c * S:(c + 1) * S] = full
        outs[d] = h
    return outs[0], outs[1], outs[2]
